# revision 40
# baseline (speedup 1.0000x reference)
"""Differential attention (B=2, T=2048, C=2048, 8 heads x 256) on 8 trn2 cores.

Sharding: tensor-parallel over the 8 effective heads — core h computes head h's
projections + attention and a partial output projection; host sums bf16
partials in f32.

Per-core pipeline (fp8 DoubleRow matmuls with hi/lo error compensation for the
projections and output projection; bf16 for QK^T and PV):
  xh/xl    [C, B*T]  fp8 e4m3 of x.T*16 (hi) and its residual (lo)
  wqvh/l   [C, 512]  head slice of [wq|wv].T * 1024, fp8 hi/lo
  wkh/l    [128, 16, 256]  head slice of wk.T * 1024 (pre-swizzled), fp8 hi/lo
  woth/l   [256, C]  head slice of wo.T * 0.2 * 4096, fp8 hi/lo
  A @ B is computed as Ah@Bh + Al@Bh + Ah@Bl, each pair of 128-contraction
  tiles packed into one DoubleRow matmul (0.5 cycles/row) -> 0.75x the bf16
  cost with ~bf16 accuracy. Scales are powers of two, folded into the existing
  descale points (rms-norm Rsqrt scale, V/KT psum evacuation, out-proj copy).

Emission is interleaved into units (proj chunk / attention q-chunk / out-proj
block pair) so the in-order PE stream always has ready matmuls; the PE
transposes trailing each unit are deferred one unit ("pending") so their
ACT/DVE producer chains overlap earlier matmul groups.

Attention math: scores computed transposed (S.T[kk,q] = K_tile.T @ Q), exp on
ACT with per-partition kscale, P.T tiles feed PV as lhsT, ones-column on V
gives the softmax denominator. Causal blocks skipped; diagonal masked
multiplicatively post-exp.
"""

import math
from contextlib import ExitStack

import numpy as np

# ---- problem constants (hardcoded per the harness contract) ----
B = 2
T = 2048
C = 2048
N_HEAD = 8
HEAD_DIM = 256
HALF = 128
LAMBDA_INIT = 0.8
RMS_EPS = 1.1920929e-07
N_CORES = 8

P = 128          # partitions
TOK_CHUNK = 512  # projection tok chunk (DMA granularity)

# fp8 scale plan (all powers of two; see module docstring)
SX = 16.0        # x -> x*SX before e4m3
SW = 1024.0      # wq|wv|wk -> *SW
SWO = 4096.0     # wo*0.2 -> *SWO
SY = 8.0         # y -> y*SY at the hi/lo split
QV_DESCALE = 1.0 / (SX * SW)              # 2^-14, applied at V/KT evacuation
RMS_SSQ_SCALE = QV_DESCALE * QV_DESCALE   # 2^-28, folded into rms Rsqrt scale
OUT_DESCALE = 1.0 / (SY * SWO)            # 2^-15, applied at out-proj copy

DEFAULT_OPTS = dict(
    att_chunk=512,       # attention q-chunk width (256 or 512)
    qk_tr="pe",          # "pe" | "dma": Q/K transpose path
    oproj_copy="alt",    # out-proj PSUM->SBUF evacuation: "act"|"dve"|"alt"
    psum=(3, 3, 2),      # banks: (proj, st, y) — must sum to <= 8
    pt_bufs=26,           # P.T tile double-buffer depth
    xc_bufs=3,           # x chunk prefetch depth
    vcopy="dve",         # "act" | "dve": V PSUM->SBUF descale-copy engine
    osb_merge=True,      # one output-store DMA per tok block (vs per c-chunk)
    narrow_top=True,     # compute only the valid half of the top causal row
    tr_pool="st",        # "st" | "pp": PSUM pool used by PE transposes
    tail_split=True,    # stream the final block's stores per c-chunk
    qn_bufs=12,
    y0_mult=2,
    ksq_eng="dve",       # "act" | "dve": engine computing k^2
    ktcopy_eng="dve",    # "act" | "dve": engine evacuating KT psum
    ytr_pool="y",        # "st" | "y": PSUM pool for the y transposes
    out_queue="sync",    # "sync" | "scalar": HWDGE queue for output stores
    prefetch=6,
    warmup=0,
    rampfill=12,
    pv_split=1,   # PV pass A covers jj < split; pass B the rest          # unit lookahead for x chunk DMA issue
    op_copy_attn=("pool", "dve", "act"),   # attn-payload oproj copy engines
    op_copy_trail=("act", "dve", "pool"),  # trailing-oproj copy engines
)


PHASES = []      # (label, first_instruction_id) marks recorded during build


def build_nc(c_dim, t_dim, b_dim, **opts):
    """Build the per-core Bass module. All shapes in tokens/channels."""
    import concourse.mybir as mybir
    import concourse.tile as tile
    from concourse import bacc
    from concourse.masks import make_identity, make_upper_triangular

    o = dict(DEFAULT_OPTS)
    o.update(opts)
    QCH = o["att_chunk"]
    jpc = QCH // P  # j-blocks per attention chunk

    dt = mybir.dt
    f32 = dt.float32
    bf16 = dt.bfloat16
    f8 = dt.float8e4
    AF = mybir.ActivationFunctionType
    OP = mybir.AluOpType
    DR = mybir.MatmulPerfMode.DoubleRow

    n_ctiles = c_dim // P            # contraction tiles over C
    n_cpairs = n_ctiles // 2         # DoubleRow c-tile pairs
    ntok = b_dim * t_dim             # total token rows
    n_blocks_b = t_dim // P          # 128-tok blocks per batch
    n_qchunks = t_dim // QCH         # attention q chunks per batch
    blocks_per_chunk = TOK_CHUNK // P
    n_chunks = t_dim // TOK_CHUNK
    VP = 272                         # V tile pitch (256 vals + 1 ones + pad)

    nc = bacc.Bacc()

    def mark(label):
        PHASES.append((label, nc.next_id()))

    xh = nc.declare_dram_parameter("xh", [c_dim, ntok], f8, isOutput=False)
    xl = nc.declare_dram_parameter("xl", [c_dim, ntok], f8, isOutput=False)
    wqvh = nc.declare_dram_parameter("wqvh", [c_dim, 512], f8, isOutput=False)
    wqvl = nc.declare_dram_parameter("wqvl", [c_dim, 512], f8, isOutput=False)
    # wk arrives pre-swizzled: [128 partitions, 16 ctiles, 256] row-major so a
    # partition's whole slab is one contiguous 4KB descriptor.
    wkh = nc.declare_dram_parameter("wkh", [P, n_ctiles * 256], f8, isOutput=False)
    wkl = nc.declare_dram_parameter("wkl", [P, n_ctiles * 256], f8, isOutput=False)
    woth = nc.declare_dram_parameter("woth", [HEAD_DIM, c_dim], f8, isOutput=False)
    wotl = nc.declare_dram_parameter("wotl", [HEAD_DIM, c_dim], f8, isOutput=False)
    lamneg = nc.declare_dram_parameter("lamneg", [P, 1], f32, isOutput=False)
    out = nc.declare_dram_parameter("out", [ntok, c_dim], bf16, isOutput=True)

    xh_r = xh.ap().rearrange("(i p) t -> p i t", p=P)      # [128, n_ctiles, ntok]
    xl_r = xl.ap().rearrange("(i p) t -> p i t", p=P)
    wqvh_r = wqvh.ap().rearrange("(i p) n -> p i n", p=P)  # [128, n_ctiles, 512]
    wqvl_r = wqvl.ap().rearrange("(i p) n -> p i n", p=P)
    wkh_r = wkh.ap().rearrange("p (i n) -> p i n", n=256)  # [128, n_ctiles, 256]
    wkl_r = wkl.ap().rearrange("p (i n) -> p i n", n=256)
    woth_r = woth.ap().rearrange("(e p) n -> p e n", p=P)  # [128, 2, c_dim]
    wotl_r = wotl.ap().rearrange("(e p) n -> p e n", p=P)

    with tile.TileContext(nc) as tc:
        with ExitStack() as ctx:
            # ---- persistent SBUF ----
            const_pool = ctx.enter_context(tc.tile_pool(name="const", bufs=1))
            wqvh_sb = const_pool.tile([P, n_ctiles, 512], f8, name="wqvh_sb")
            wqvl_sb = const_pool.tile([P, n_ctiles, 512], f8, name="wqvl_sb")
            wkh_sb = const_pool.tile([P, n_ctiles, 256], f8, name="wkh_sb")
            wkl_sb = const_pool.tile([P, n_ctiles, 256], f8, name="wkl_sb")
            woth_sb = const_pool.tile([P, 2, c_dim], f8, name="woth_sb")
            wotl_sb = const_pool.tile([P, 2, c_dim], f8, name="wotl_sb")
            lam_sb = const_pool.tile([P, 1], f32, name="lam_sb")
            ident = const_pool.tile([P, P], bf16, name="ident")
            trimask = const_pool.tile([P, P], bf16, name="trimask")
            ones_sb = const_pool.tile([P, 1], bf16, name="ones_sb")
            nc.vector.memset(ones_sb[:], 1.0)

            xch_pool = ctx.enter_context(tc.tile_pool(name="xch", bufs=o["xc_bufs"]))
            xcl_pool = ctx.enter_context(tc.tile_pool(name="xcl", bufs=o["xc_bufs"]))

            # ---- ramp: one SP-queue stream in exact consumption order so
            # first-chunk matmuls start on partial data; qv weight halves are
            # interleaved into the K stream so QV matmuls can fill the
            # DMA-bound holes of the K phase. ----
            xc0h = xch_pool.tile([P, n_ctiles, TOK_CHUNK], f8, tag="xch",
                                 name="xc0h")
            xc0l = xcl_pool.tile([P, n_ctiles, TOK_CHUNK], f8, tag="xcl",
                                 name="xc0l")
            nc.sync.dma_start(wkh_sb[:], wkh_r[:])
            nc.sync.dma_start(xc0h[:, 0:8, :], xh_r[:, 0:8, 0:TOK_CHUNK])
            nc.sync.dma_start(wqvh_sb[:, 0:8, :], wqvh_r[:, 0:8, :])
            nc.sync.dma_start(xc0h[:, 8:16, :], xh_r[:, 8:16, 0:TOK_CHUNK])
            nc.sync.dma_start(wkl_sb[:], wkl_r[:])
            nc.sync.dma_start(wqvh_sb[:, 8:16, :], wqvh_r[:, 8:16, :])
            nc.sync.dma_start(xc0l[:, 0:8, :], xl_r[:, 0:8, 0:TOK_CHUNK])
            nc.sync.dma_start(wqvl_sb[:, 0:8, :], wqvl_r[:, 0:8, :])
            nc.sync.dma_start(xc0l[:, 8:16, :], xl_r[:, 8:16, 0:TOK_CHUNK])
            nc.sync.dma_start(wqvl_sb[:, 8:16, :], wqvl_r[:, 8:16, :])
            nc.scalar.dma_start(lam_sb[:], lamneg.ap())
            make_identity(nc, ident[:])
            # 1.0 where kk <= q (partition <= free), else 0
            make_upper_triangular(nc, trimask[:], val=1.0, diag=True)

            qt_pool = ctx.enter_context(tc.tile_pool(name="qt", bufs=2))
            kt_pool = ctx.enter_context(tc.tile_pool(name="kt", bufs=2))
            ksq_pool = ctx.enter_context(tc.tile_pool(name="ksq", bufs=2))
            kscale_pool = ctx.enter_context(tc.tile_pool(name="kscale", bufs=2))
            v_pool = ctx.enter_context(tc.tile_pool(name="v", bufs=2))
            yth_pool = ctx.enter_context(tc.tile_pool(name="yth", bufs=5))
            ytl_pool = ctx.enter_context(tc.tile_pool(name="ytl", bufs=5))
            pt_pool = ctx.enter_context(tc.tile_pool(name="pt", bufs=o["pt_bufs"]))
            y0_pool = ctx.enter_context(tc.tile_pool(name="y0", bufs=o["y0_mult"] * jpc))
            osb_pool = ctx.enter_context(tc.tile_pool(name="osb", bufs=3))
            qn_pool = ctx.enter_context(tc.tile_pool(name="qn", bufs=o["qn_bufs"]))
            sq_pool = ctx.enter_context(tc.tile_pool(name="sq", bufs=2))
            qcp_pool = ctx.enter_context(tc.tile_pool(name="qcp", bufs=8))
            rms_pool = ctx.enter_context(tc.tile_pool(name="rms", bufs=12))
            nproj, nst, ny = o["psum"]
            psum_proj = ctx.enter_context(
                tc.tile_pool(name="psum_proj", bufs=nproj, space="PSUM"))
            psum_st = ctx.enter_context(
                tc.tile_pool(name="psum_st", bufs=nst, space="PSUM"))
            psum_y = ctx.enter_context(
                tc.tile_pool(name="psum_y", bufs=ny, space="PSUM"))

            tr_psum = psum_st if o["tr_pool"] == "st" else psum_proj
            tr_tag = o["tr_pool"] if o["tr_pool"] == "st" else "pp"
            tr_shape = 256 if o["tr_pool"] == "st" else 512

            # PE p-state warm-up: keep the tensor engine continuously busy
            # with junk transposes while the first DMAs land, so the real
            # matmuls start at full clock (cost model p-state ramp).
            for _ in range(o["warmup"]):
                wtrp = tr_psum.tile([P, tr_shape], bf16, tag=tr_tag,
                                    name="wtrp")[:, :P]
                nc.tensor.transpose(wtrp, ident[:], ident[:])

            def pe_transpose(dst_ap, src_ap):
                trp = tr_psum.tile([P, tr_shape], bf16, tag=tr_tag,
                                   name="trp")[:, :P]
                nc.tensor.transpose(trp, src_ap, ident[:])
                nc.vector.tensor_copy(dst_ap, trp)

            tr_qk = pe_transpose

            # ---------------- per-batch tile state ----------------
            bt = {}

            def batch_tiles(b):
                if b not in bt:
                    bt[b] = dict(
                        qt=qt_pool.tile([P, 2, t_dim], bf16, name=f"qt_b{b}", tag="qt"),
                        kt=kt_pool.tile([P, 2, t_dim], bf16, name=f"kt_b{b}", tag="kt"),
                        v=v_pool.tile([P, n_blocks_b, VP], bf16, name=f"v_b{b}", tag="v"),
                        ksc=kscale_pool.tile([P, 2 * n_blocks_b], f32,
                                             name=f"ksc_b{b}", tag="ksc"),
                    )
                return bt[b]

            pending = []     # deferred emitters: (enqueue_unit_idx, fn)
            cur_unit = [0]

            def flush_pending(min_age=2):
                keep = []
                for enq, fn in pending:
                    if cur_unit[0] - enq >= min_age:
                        fn()
                    else:
                        keep.append((enq, fn))
                pending[:] = keep

            # x chunk DMA management (prefetched ahead of the unit stream)
            xc_tiles = {(0, 0): (xc0h, xc0l)}
            ytp = {}     # (b, attn chunk) -> per-chunk (yth, ytl) fp8 tiles

            def issue_xc(b, ch):
                if (b, ch) in xc_tiles:
                    return
                tok0 = b * t_dim + ch * TOK_CHUNK
                xch = xch_pool.tile([P, n_ctiles, TOK_CHUNK], f8, tag="xch")
                xcl = xcl_pool.tile([P, n_ctiles, TOK_CHUNK], f8, tag="xcl")
                nc.sync.dma_start(xch[:], xh_r[:, :, tok0:tok0 + TOK_CHUNK])
                nc.sync.dma_start(xcl[:], xl_r[:, :, tok0:tok0 + TOK_CHUNK])
                xc_tiles[(b, ch)] = (xch, xcl)

            # ---------------- unit emitters ----------------
            def proj_segments(b, ch):
                """Split one projection chunk into ~2.5us emission segments so
                attention units can interleave them as PE filler."""
                t_ = batch_tiles(b)
                kt_sb, qt_sb, v_sb, ksc_sb = t_["kt"], t_["qt"], t_["v"], t_["ksc"]
                xch, xcl = xc_tiles.pop((b, ch))
                st_ = {"ksqs": [], "qcps": [], "rmsg": None}

                def seg_k_tail(v, ktp):
                    ktdst = kt_sb[:, v, ch * TOK_CHUNK:(ch + 1) * TOK_CHUNK]
                    if o["ktcopy_eng"] == "act":
                        nc.scalar.activation(ktdst, ktp[:], AF.Copy,
                                             scale=QV_DESCALE)
                    else:
                        nc.vector.tensor_scalar_mul(ktdst, ktp[:], QV_DESCALE)
                    ksq = ksq_pool.tile([P, TOK_CHUNK], bf16, tag="ksq")
                    if o["ksq_eng"] == "act":
                        nc.scalar.activation(ksq[:], ktdst, AF.Square)
                    elif o["ksq_eng"] == "pool":
                        nc.gpsimd.tensor_tensor(ksq[:], ktdst, ktdst,
                                                op=OP.mult)
                    else:
                        nc.vector.tensor_tensor(ksq[:], ktdst, ktdst, op=OP.mult)
                    st_["ksqs"].append(ksq)

                def seg_k(v):
                    if v == 0:
                        mark(f"b{b}_proj_ch{ch}")
                    ktp = psum_proj.tile([P, 512], f32, tag="pp", name="ktp")
                    idx = 0
                    for X, W in ((xch, wkh_sb), (xch, wkl_sb), (xcl, wkh_sb)):
                        for pi in range(n_cpairs):
                            nc.tensor.matmul(
                                ktp[:],
                                W[:, 2 * pi:2 * pi + 2, v * P:(v + 1) * P],
                                X[:, 2 * pi:2 * pi + 2, :],
                                start=(idx == 0),
                                stop=(idx == 3 * n_cpairs - 1),
                                perf_mode=DR)
                            idx += 1
                    seg_k_tail(v, ktp)

                def seg_ramp():
                    # First chunk: K and QV matmuls interleaved in DMA stream
                    # order (wkh, xh1, wqvh1, xh2, wkl, wqvh2, xl1, wqvl1,
                    # xl2, wqvl2) so each matmul gates only on the half-DMAs
                    # it reads and QV work fills the K phase's DMA holes.
                    # Extra accumulators borrow the (still idle) st psum pool.
                    mark(f"b{b}_proj_ch{ch}")
                    for _ in range(o["rampfill"]):   # warm p-state pre-data
                        wtrp = tr_psum.tile([P, tr_shape], bf16,
                                            tag=tr_tag, name="wtrp")[:, :P]
                        nc.tensor.transpose(wtrp, ident[:], ident[:])
                    st_["rmsg"] = rms_pool.tile([P, 8], f32, tag="rms",
                                                name="rmsg")
                    ktps = [psum_proj.tile([P, 512], f32, tag="pp",
                                           name=f"ktp{v}") for v in range(2)]
                    qvs = [psum_st.tile([P, 512], f32, tag="st",
                                        name=f"qvr{tl}") for tl in range(3)]
                    qvs.append(psum_proj.tile([P, 512], f32, tag="pp",
                                              name="qvr3"))
                    kidx = [0, 0]
                    qidx = [0, 0, 0, 0]

                    def kmm(X, W, pi):
                        for v in range(2):
                            nc.tensor.matmul(
                                ktps[v][:],
                                W[:, 2 * pi:2 * pi + 2, v * P:(v + 1) * P],
                                X[:, 2 * pi:2 * pi + 2, :],
                                start=(kidx[v] == 0),
                                stop=(kidx[v] == 3 * n_cpairs - 1),
                                perf_mode=DR)
                            kidx[v] += 1

                    def qmm(X, W, pi, tl):
                        nc.tensor.matmul(
                            qvs[tl][:],
                            X[:, 2 * pi:2 * pi + 2, tl * P:(tl + 1) * P],
                            W[:, 2 * pi:2 * pi + 2, 0:512],
                            start=(qidx[tl] == 0),
                            stop=(qidx[tl] == 3 * n_cpairs - 1),
                            perf_mode=DR)
                        qidx[tl] += 1

                    for pi in range(4):                       # wkh + xh1
                        kmm(xch, wkh_sb, pi)
                    for tl in range(4):                       # wqvh1
                        for pi in range(4):
                            qmm(xch, wqvh_sb, pi, tl)
                    for pi in range(4, 8):                    # xh2
                        kmm(xch, wkh_sb, pi)
                    for pi in range(8):                       # wkl
                        kmm(xch, wkl_sb, pi)
                    for tl in range(4):                       # wqvh2
                        for pi in range(4, 8):
                            qmm(xch, wqvh_sb, pi, tl)
                    for pi in range(4):                       # xl1
                        kmm(xcl, wkh_sb, pi)
                    for tl in range(4):
                        for pi in range(4):
                            qmm(xcl, wqvh_sb, pi, tl)
                    for tl in range(4):                       # wqvl1
                        for pi in range(4):
                            qmm(xch, wqvl_sb, pi, tl)
                    for pi in range(4, 8):                    # xl2
                        kmm(xcl, wkh_sb, pi)
                    for tl in range(4):
                        for pi in range(4, 8):
                            qmm(xcl, wqvh_sb, pi, tl)
                    for v in range(2):
                        seg_k_tail(v, ktps[v])
                    for tl in range(4):                       # wqvl2
                        for pi in range(4, 8):
                            qmm(xch, wqvl_sb, pi, tl)
                        qv_tail(tl, qvs[tl])

                def qv_tail(tl, qv):
                    rmsg = st_["rmsg"]
                    tb = ch * blocks_per_chunk + tl
                    for j in range(2):
                        sq = sq_pool.tile([P, P], bf16, tag="sq", name="sq")
                        nc.scalar.activation(sq[:], qv[:, j * P:(j + 1) * P],
                                             AF.Square,
                                             accum_out=rmsg[:, 2 * tl + j:
                                                            2 * tl + j + 1])
                    qcp = qcp_pool.tile([P, 256], bf16, tag="qcp")
                    nc.vector.tensor_scalar_mul(qcp[:], qv[:, 0:256],
                                                QV_DESCALE)
                    st_["qcps"].append(qcp)
                    # V (+ ones column for the softmax denominator)
                    if o["vcopy"] == "act":
                        nc.scalar.activation(v_sb[:, tb, 0:256], qv[:, 256:512],
                                             AF.Copy, scale=QV_DESCALE)
                    else:
                        nc.vector.tensor_scalar_mul(v_sb[:, tb, 0:256],
                                                    qv[:, 256:512], QV_DESCALE)
                    nc.vector.memset(v_sb[:, tb, 256:257], 1.0)

                def seg_qv(tl):
                    # sq accumulates the whole chunk's ssq into one [P,8] tile;
                    # a descaled q copy (qcp) releases the psum early. Newton
                    # rsqrt, qn muls and PE transposes are deferred.
                    if tl == 0:
                        st_["rmsg"] = rms_pool.tile([P, 8], f32, tag="rms", name="rmsg")
                    qv = psum_proj.tile([P, 512], f32, tag="pp", name="qv")
                    idx = 0
                    for X, W in ((xch, wqvh_sb), (xcl, wqvh_sb), (xch, wqvl_sb)):
                        for pi in range(n_cpairs):
                            nc.tensor.matmul(
                                qv[:],
                                X[:, 2 * pi:2 * pi + 2, tl * P:(tl + 1) * P],
                                W[:, 2 * pi:2 * pi + 2, 0:512],
                                start=(idx == 0),
                                stop=(idx == 3 * n_cpairs - 1),
                                perf_mode=DR)
                            idx += 1
                    qv_tail(tl, qv)

                def newton8(dst, m, entry_scale, entry_bias, post=None):
                    """dst = 1/sqrt(m*scale + bias) via seed + 2 Newton iters
                    (DVE-only; ACT Sqrt would force act-table swaps)."""
                    km = rms_pool.tile([P, 8], f32, tag="rms", name="nm")
                    nc.vector.tensor_scalar(km[:], m, entry_scale, entry_bias,
                                            OP.mult, OP.add)
                    t1 = rms_pool.tile([P, 8], f32, tag="rms", name="nt")
                    nc.vector.tensor_tensor(t1[:], km[:], km[:], op=OP.mult)
                    nc.vector.tensor_scalar(dst, km[:], -1.47991565,
                                            2.07556761, OP.mult, OP.add)
                    nc.vector.scalar_tensor_tensor(
                        dst, t1[:], 0.41306651, dst, op0=OP.mult, op1=OP.add)
                    nc.vector.tensor_scalar_max(dst, dst, 0.05)
                    for _ in range(2):
                        nc.vector.tensor_tensor(t1[:], dst, dst, op=OP.mult)
                        nc.vector.scalar_tensor_tensor(
                            t1[:], t1[:], -0.5, km[:], op0=OP.mult, op1=OP.mult)
                        nc.vector.tensor_scalar(t1[:], t1[:], 1.0, 1.5,
                                                OP.mult, OP.add)
                        nc.vector.tensor_tensor(dst, dst, t1[:], op=OP.mult)
                    if post is not None:
                        nc.vector.tensor_scalar_mul(dst, dst, post)

                def seg_tail():
                    # kssq matmuls + kscale (per-block k rms)
                    kssq = psum_proj.tile([P, 512], f32, tag="pp",
                                          name="kssq")[:, :8]
                    for v in range(2):
                        for t in range(blocks_per_chunk):
                            nc.tensor.matmul(
                                kssq[:, 2 * t + v:2 * t + v + 1],
                                st_["ksqs"][v][:, t * P:(t + 1) * P],
                                ones_sb[:], start=True, stop=True)
                    ksl = ksc_sb[:, ch * 2 * blocks_per_chunk:
                                 (ch + 1) * 2 * blocks_per_chunk]
                    newton8(ksl, kssq[:], 1.0 / HALF, RMS_EPS,
                            post=1.0 / math.sqrt(HALF))
                    # q rms: 1/sqrt(ssq * 2^-28 / 128 + eps) (qcp holds true q)
                    yv = rms_pool.tile([P, 8], f32, tag="rms", name="yv")
                    newton8(yv[:], st_["rmsg"][:], RMS_SSQ_SCALE / HALF,
                            RMS_EPS)
                    qcps = st_["qcps"]

                    def defer_fn(ch=ch, qcps=qcps, yv=yv):
                        for tl in range(blocks_per_chunk):
                            tb = ch * blocks_per_chunk + tl
                            for j in range(2):
                                qn = qn_pool.tile([P, P], bf16, tag="qn")
                                nc.vector.tensor_scalar_mul(
                                    qn[:], qcps[tl][:, j * P:(j + 1) * P],
                                    yv[:, 2 * tl + j:2 * tl + j + 1])
                                if o["qk_tr"] == "dma":
                                    nc.scalar.dma_start_transpose(
                                        out=qt_sb[:, j, tb * P:(tb + 1) * P],
                                        in_=qn[:])
                                else:
                                    tr_qk(qt_sb[:, j, tb * P:(tb + 1) * P],
                                          qn[:])
                    pending.append((cur_unit[0], defer_fn))

                if b == 0 and ch == 0:
                    return [seg_ramp, seg_tail]
                return ([lambda: seg_k(0), lambda: seg_k(1)]
                        + [lambda tl=tl: seg_qv(tl)
                           for tl in range(blocks_per_chunk)]
                        + [seg_tail])

            def emit_proj(b, ch):
                segs = proj_segments(b, ch)
                segs[0]()
                flush_pending(min_age=1)
                for fn in segs[1:]:
                    fn()

            def make_op_groups(b, tb_list, stream=False, tail=False,
                               copy_eng=None):
                """Per-(tb,cc) out-proj emitters: 3 DoubleRow matmuls + one
                PSUM->SBUF bf16 copy (engine per copy_eng / oproj_copy opt;
                "pool" = gpsimd, freeing ACT/DVE); the tok-block's store DMA
                rides the last cc (or one DMA per cc when streaming the
                drain). Out values stay scaled by SY*SWO; the host applies
                OUT_DESCALE after the cross-core sum."""
                groups = []
                for tb in tb_list:
                    orow = osb_pool.tile([P, c_dim], bf16, tag="orow",
                                         name="orow")
                    for cc in range(c_dim // 512):
                        def g(tb=tb, cc=cc, orow=orow, b=b, stream=stream,
                              tail=tail, copy_eng=copy_eng):
                            row0 = b * t_dim + tb * P
                            yth_t, ytl_t = ytp[(b, tb // jpc)]
                            jj = tb % jpc
                            op_ps = psum_proj.tile([P, 512], f32, tag="pp",
                                                   name="ops")
                            tsl = slice(jj * P, (jj + 1) * P)
                            csl = slice(cc * 512, (cc + 1) * 512)
                            nc.tensor.matmul(op_ps[:], yth_t[:, 0:2, tsl],
                                             woth_sb[:, 0:2, csl],
                                             start=True, stop=False,
                                             perf_mode=DR)
                            nc.tensor.matmul(op_ps[:], ytl_t[:, 0:2, tsl],
                                             woth_sb[:, 0:2, csl],
                                             start=False, stop=False,
                                             perf_mode=DR)
                            nc.tensor.matmul(op_ps[:], yth_t[:, 0:2, tsl],
                                             wotl_sb[:, 0:2, csl],
                                             start=False, stop=True,
                                             perf_mode=DR)
                            osb = orow[:, cc * 512:(cc + 1) * 512]
                            oc = copy_eng or o["oproj_copy"]
                            if isinstance(oc, tuple):
                                oc = oc[(4 * tb + cc) % len(oc)]
                            if oc == "alt":
                                oc = "act" if (tb + cc) % 2 == 0 else "dve"
                            if oc == "act":
                                nc.scalar.activation(osb, op_ps[:], AF.Copy)
                            elif oc == "pool":
                                nc.gpsimd.tensor_copy(osb, op_ps[:])
                            else:
                                nc.vector.tensor_copy(osb, op_ps[:])
                            out_eng = (nc.sync if o["out_queue"] == "sync"
                                       else nc.scalar)
                            percc = stream and (stream == "all"
                                                or tb == n_blocks_b - 1)
                            if percc:
                                out_eng.dma_start(
                                    out.ap()[row0:row0 + P,
                                             cc * 512:(cc + 1) * 512], osb)
                            elif cc == c_dim // 512 - 1:
                                out_eng.dma_start(
                                    out.ap()[row0:row0 + P, :], orow[:])
                        groups.append(g)
                return groups

            def emit_attn(b, cqi, op_list=(), fillers=(), drain=False):
                mark(f"b{b}_attn_c{cqi}")
                # attn00 runs right after proj00 inside the xc1 DMA window;
                # its qt/kt chain must flush eagerly (age-1 pending). The
                # drain unit also flushes eagerly so the prior chunk's y is
                # ready for payload groups from the unit start.
                eager = (b == 0 and cqi == 0) or drain
                flush_pending(min_age=1 if eager else 2)
                op_groups = []   # (eligible_frac, fn) — late payloads gated
                for entry in op_list:
                    ob, blocks = entry[0], entry[1]
                    sfrac = entry[2] if len(entry) > 2 else 0.0
                    ceng = entry[3] if len(entry) > 3 else o["op_copy_attn"]
                    gs = make_op_groups(ob, blocks, copy_eng=ceng)
                    for k, g in enumerate(gs):
                        op_groups.append(
                            (sfrac + (1.0 - sfrac) * k / len(gs), g))
                op_groups.sort(key=lambda t: t[0])
                gi = 0
                fi = 0
                jmax_ = jpc * cqi + (jpc - 1)
                # pass A weighted 3x; drain mode runs v1's PV per-jj
                n_iters = (4 + (3 + jpc - 1 if drain else 4)) * (jmax_ + 1)
                it = 0

                def sprinkle():
                    nonlocal gi, fi
                    while (gi < len(op_groups)
                           and op_groups[gi][0] * n_iters <= it):
                        op_groups[gi][1]()
                        gi += 1
                    while (fi < len(fillers)
                           and fi + 1 <= len(fillers) * it // n_iters):
                        fillers[fi]()
                        fi += 1

                t_ = batch_tiles(b)
                kt_sb, qt_sb, v_sb, ksc_sb = t_["kt"], t_["qt"], t_["v"], t_["ksc"]
                q0 = cqi * QCH
                jmax = jmax_
                half = o["pv_split"]
                y0s = {}
                yfs = []

                def drain_block(jj):
                    # last chunk: transpose + out-proj + per-cc stores for
                    # this query block immediately, inside the unit
                    if (b, cqi) not in ytp:
                        ytp[(b, cqi)] = (
                            yth_pool.tile([P, 2, QCH], f8, tag="yth",
                                          name="yth_d"),
                            ytl_pool.tile([P, 2, QCH], f8, tag="ytl",
                                          name="ytl_d"))
                    yth_t, ytl_t = ytp[(b, cqi)]
                    j = jpc * cqi + jj
                    yf = next(f for jf, f in yfs if jf == j)
                    for e in range(2):
                        trp = psum_y.tile([P, 257], bf16, tag="y",
                                          name="trpy")[:, :P]
                        nc.tensor.transpose(trp, yf[:, e * P:(e + 1) * P],
                                            ident[:])
                        hdst = yth_t[:, e, jj * P:(jj + 1) * P]
                        ldst = ytl_t[:, e, jj * P:(jj + 1) * P]
                        nc.vector.tensor_scalar_mul(hdst, trp, SY)
                        nc.vector.scalar_tensor_tensor(
                            ldst, trp, SY, hdst,
                            op0=OP.mult, op1=OP.subtract)
                    for g in make_op_groups(b, [jpc * cqi + jj], stream="all",
                                            tail=True,
                                            copy_eng=o["op_copy_trail"]):
                        g()

                for v in range(2):
                    pts = []
                    vpasses = ([(jj, jj + 1) for jj in range(jpc)]
                               if drain and v == 1
                               else ((0, half), (half, jpc)))
                    for jj_lo, jj_hi in vpasses:
                        ys = {jj: psum_y.tile([P, 257], f32, tag="y", name="ys")
                              for jj in range(jj_lo, jj_hi)}
                        for i in range(jmax + 1):
                            if jj_lo == 0:
                                # pass A: compute scores + exp as we go
                                jj0 = (max(0, i - jpc * cqi)
                                       if o["narrow_top"] else 0)
                                w = QCH - jj0 * P
                                st = psum_st.tile([P, QCH], f32, tag="st",
                                                  name="st")[:, :w]
                                nc.tensor.matmul(
                                    st[:], kt_sb[:, v, i * P:(i + 1) * P],
                                    qt_sb[:, v, q0 + jj0 * P:q0 + QCH],
                                    start=True, stop=True)
                                pt = pt_pool.tile([P, QCH], bf16, tag="pt",
                                                  name="pt")[:, :w]
                                nc.scalar.activation(
                                    pt[:], st[:], AF.Exp,
                                    scale=ksc_sb[:, 2 * i + v:2 * i + v + 1])
                                dj = i - jpc * cqi
                                if dj >= 0:
                                    nc.vector.tensor_tensor(
                                        pt[:, (dj - jj0) * P:(dj - jj0 + 1) * P],
                                        pt[:, (dj - jj0) * P:(dj - jj0 + 1) * P],
                                        trimask[:], op=OP.mult)
                                pts.append((pt, jj0))
                            else:
                                pt, jj0 = pts[i]
                            for jj in range(max(jj_lo, jj0), jj_hi):
                                j = jpc * cqi + jj
                                if i > j:
                                    continue
                                nc.tensor.matmul(
                                    ys[jj][:],
                                    pt[:, (jj - jj0) * P:(jj - jj0 + 1) * P],
                                    v_sb[:, i, 0:257],
                                    start=(i == 0), stop=(i == j))
                            it += 1 if jj_lo != 0 else 3
                            sprinkle()
                        if jj_lo == 0:
                            flush_pending(min_age=1)
                        # epilogue for this pass/view
                        for jj in range(jj_lo, jj_hi):
                            j = jpc * cqi + jj
                            inv = rms_pool.tile([P, 1], f32, tag="inv")
                            nc.vector.reciprocal(inv[:], ys[jj][:, 256:257])
                            if v == 0:
                                y0 = y0_pool.tile([P, 256], f32, tag="y0")
                                nc.vector.tensor_scalar_mul(
                                    y0[:], ys[jj][:, 0:256], inv[:])
                                y0s[jj] = y0
                            else:
                                sc2 = rms_pool.tile([P, 1], f32, tag="inv")
                                nc.vector.tensor_tensor(
                                    sc2[:], inv[:], lam_sb[:], op=OP.mult)
                                yf = qn_pool.tile([P, 256], bf16, tag="yf")
                                nc.vector.scalar_tensor_tensor(
                                    yf[:], ys[jj][:, 0:256], sc2[:],
                                    y0s[jj][:], op0=OP.mult, op1=OP.add)
                                yfs.append((j, yf))
                        if drain and v == 1 and jj_lo >= 1:
                            # one-pass lag so the epilogue DVE chain hides
                            # behind the next block's PV matmuls
                            drain_block(jj_lo - 1)
                    if drain and v == 1:
                        drain_block(jpc - 1)
                while gi < len(op_groups):
                    op_groups[gi][1]()
                    gi += 1
                while fi < len(fillers):
                    fillers[fi]()
                    fi += 1

                def defer_fn(yfs=yfs, b=b, cqi=cqi):
                    yth_t = yth_pool.tile([P, 2, QCH], f8, tag="yth")
                    ytl_t = ytl_pool.tile([P, 2, QCH], f8, tag="ytl")
                    ytp[(b, cqi)] = (yth_t, ytl_t)
                    for j, yf in yfs:
                        jj = j - jpc * cqi
                        for e in range(2):
                            if o["ytr_pool"] == "y":
                                trp = psum_y.tile([P, 257], bf16, tag="y",
                                                  name="trpy")[:, :P]
                            else:
                                trp = tr_psum.tile([P, tr_shape], bf16,
                                                   tag=tr_tag, name="trp")[:, :P]
                            nc.tensor.transpose(trp, yf[:, e * P:(e + 1) * P],
                                                ident[:])
                            hdst = yth_t[:, e, jj * P:(jj + 1) * P]
                            ldst = ytl_t[:, e, jj * P:(jj + 1) * P]
                            nc.vector.tensor_scalar_mul(hdst, trp, SY)
                            nc.vector.scalar_tensor_tensor(
                                ldst, trp, SY, hdst,
                                op0=OP.mult, op1=OP.subtract)
                if not drain:   # drain mode transposed + projected inline
                    pending.append((cur_unit[0], defer_fn))

            def emit_oproj(b, tb_lo, tb_hi):
                mark(f"b{b}_oproj_tb{tb_lo}")
                flush_pending(min_age=1)
                groups = make_op_groups(b, range(tb_lo, tb_hi),
                                        stream=o["tail_split"], tail=True,
                                        copy_eng=o["op_copy_trail"])
                for g in groups:
                    g()

            # ---------------- unit schedule ----------------
            # unit schedule: (kind, b, arg, op_payload, fill) — op-block
            # payloads (owner batch, tok-blocks) ride attention units two-plus
            # units after the blocks' own attention chunk.
            PD = ("pool", "dve")
            ADP = ("act", "dve", "pool")
            PAD = ("pool", "act", "dve")
            units = [
                ("proj", 0, 0),
                ("attn", 0, 0, (), None),
                ("proj", 0, 1),
                ("proj", 0, 2),
                ("attn", 0, 1, (), None),
                ("proj", 0, 3),
                ("attn", 0, 2, ((0, (0, 1, 2, 3), 0.0, PD),), None),
                ("attn", 0, 3, ((0, (4, 5, 6, 7), 0.0, PD),), None),
                ("proj", 1, 0), ("proj", 1, 1),
                ("attn", 1, 0, ((0, (8, 9, 10), 0.0, ADP),), None),
                ("proj", 1, 2),
                ("attn", 1, 1, ((0, (11, 12, 13, 14, 15), 0.0, ADP),), None),
                ("proj", 1, 3),
                ("attn", 1, 2, ((1, (0, 1, 2, 3), 0.0, PD),), None),
                ("attn", 1, 3, ((1, (4, 5, 6, 7), 0.0, PAD),
                                (1, (8, 9, 10, 11), 0.08, PAD)), None),
            ]

            for idx, unit in enumerate(units):
                cur_unit[0] = idx
                # prefetch x chunks a few units ahead (in order)
                for u2 in units[idx:idx + o["prefetch"]]:
                    if u2[0] == "proj":
                        issue_xc(u2[1], u2[2])
                    elif u2[0] == "attn" and u2[4] is not None:
                        issue_xc(u2[4][0], u2[4][1])
                kind, b = unit[0], unit[1]
                if kind == "proj":
                    emit_proj(b, unit[2])
                elif kind == "attn":
                    payload, fill = unit[3], unit[4]
                    fillers = (proj_segments(fill[0], fill[1])
                               if fill is not None else ())
                    emit_attn(b, unit[2], op_list=payload, fillers=fillers,
                              drain=(b == b_dim - 1
                                     and unit[2] == n_qchunks - 1))
                else:
                    emit_oproj(b, unit[2][0], unit[2][1])
                if idx == 3:
                    # out-proj weights on the sync queue: strictly after the
                    # ramp stream and xc1-xc3 prefetch (first use: unit 6)
                    nc.sync.dma_start(woth_sb[:], woth_r[:])
                    nc.sync.dma_start(wotl_sb[:], wotl_r[:])
            cur_unit[0] += 10
            flush_pending(min_age=0)
    nc.compile()
    return nc


_NC_CACHE = {}
TRACE = False        # set True (e.g. from test.py) to capture an NTFF profile
LAST_RESULT = None   # BassKernelResults of the most recent run


def _get_nc(c_dim, t_dim, b_dim, **opts):
    key = (c_dim, t_dim, b_dim, tuple(sorted(opts.items())))
    if key not in _NC_CACHE:
        _NC_CACHE[key] = build_nc(c_dim, t_dim, b_dim, **opts)
    return _NC_CACHE[key]


def _fp8_pair(a):
    """hi + residual lo decomposition into e4m3 (values must be in range)."""
    import ml_dtypes
    E4 = ml_dtypes.float8_e4m3
    a = np.clip(a, -224.0, 224.0)
    hi = a.astype(E4)
    lo = (a - hi.astype(np.float32)).astype(E4)
    return hi, lo


def prep_inputs(x, wq, wk, wv, wo, lq1, lk1, lq2, lk2):
    """Host-side prep: per-core input maps (fp8 hi/lo operand pairs)."""
    b_dim, t_dim, c_dim = x.shape
    n_ctiles = c_dim // P

    lam1 = np.exp(np.sum(lq1.astype(np.float64) * lk1.astype(np.float64)))
    lam2 = np.exp(np.sum(lq2.astype(np.float64) * lk2.astype(np.float64)))
    lam_full = np.float32(lam1 - lam2 + LAMBDA_INIT)

    xt = np.ascontiguousarray(
        x.reshape(b_dim * t_dim, c_dim).T).astype(np.float32) * SX
    xh, xl = _fp8_pair(xt)
    lamneg = np.full((P, 1), -lam_full, dtype=np.float32)

    in_maps = []
    for h in range(N_CORES):
        sl = slice(h * HEAD_DIM, (h + 1) * HEAD_DIM)
        wqv = np.concatenate([wq[sl].T, wv[sl].T], axis=1) * SW
        wqvh, wqvl = _fp8_pair(np.ascontiguousarray(wqv))
        # wk pre-swizzled to [128, n_ctiles * 256]
        wk_h = (wk[sl].T * SW).reshape(n_ctiles, P, 256).transpose(1, 0, 2)
        wk_h = np.ascontiguousarray(wk_h).reshape(P, n_ctiles * 256)
        wkh, wkl = _fp8_pair(wk_h)
        wot = np.ascontiguousarray(
            (wo[:, sl] * ((1.0 - LAMBDA_INIT) * SWO)).T)
        woth, wotl = _fp8_pair(wot)
        in_maps.append({
            "xh": xh, "xl": xl, "wqvh": wqvh, "wqvl": wqvl,
            "wkh": wkh, "wkl": wkl, "woth": woth, "wotl": wotl,
            "lamneg": lamneg,
        })
    return in_maps


_FN_CACHE = {}


def _get_callable(nc):
    """Build (once) a reusable jitted shard_map callable for this module —
    mirrors bass2jax.run_bass_via_pjrt's multi-core path, but cached so
    repeat kernel() calls skip retracing."""
    if id(nc) in _FN_CACHE:
        return _FN_CACHE[id(nc)]
    import jax
    from jax.sharding import Mesh, PartitionSpec, NamedSharding
    from jax.experimental.shard_map import shard_map
    import concourse.mybir as mybir
    import concourse.bass2jax as b2j

    b2j.install_neuronx_cc_hook()
    pname = nc.partition_id_tensor.name if nc.partition_id_tensor else None
    in_names, out_names, out_avals, zero_shapes = [], [], [], []
    for alloc in nc.m.functions[0].allocations:
        if not isinstance(alloc, mybir.MemoryLocationSet):
            continue
        name = alloc.memorylocations[0].name
        if alloc.kind == "ExternalInput":
            if name != pname:
                in_names.append(name)
        elif alloc.kind == "ExternalOutput":
            out_names.append(name)
            shape = tuple(alloc.tensor_shape)
            dtype = mybir.dt.np(alloc.dtype)
            out_avals.append(jax.core.ShapedArray(shape, dtype))
            zero_shapes.append((shape, dtype))
    n_params = len(in_names)
    all_in = in_names + out_names
    if pname is not None:
        all_in = all_in + [pname]

    def _body(*args):
        operands = list(args)
        if pname is not None:
            operands.append(b2j.partition_id_tensor())
        return tuple(b2j._bass_exec_p.bind(
            *operands,
            out_avals=tuple(out_avals),
            in_names=tuple(all_in),
            out_names=tuple(out_names),
            lowering_input_output_aliases=(),
            sim_require_finite=True,
            sim_require_nnan=True,
            nc=nc,
        ))

    devices = jax.devices()[:N_CORES]
    mesh = Mesh(np.asarray(devices), ("core",))
    nio = n_params + len(out_names)
    fn = jax.jit(shard_map(_body, mesh=mesh,
                           in_specs=(PartitionSpec("core"),) * nio,
                           out_specs=(PartitionSpec("core"),) * len(out_names),
                           check_rep=False),
                 donate_argnums=tuple(range(n_params, nio)), keep_unused=True)
    sh = NamedSharding(mesh, PartitionSpec("core"))
    entry = (fn, in_names, out_names, zero_shapes, sh)
    _FN_CACHE[id(nc)] = entry
    return entry


def kernel(x, wq, wk, wv, wo, lq1, lk1, lq2, lk2):
    b_dim, t_dim, c_dim = x.shape
    in_maps = prep_inputs(x, wq, wk, wv, wo, lq1, lk1, lq2, lk2)
    nc = _get_nc(c_dim, t_dim, b_dim)

    try:
        import jax
        fn, in_names, out_names, zero_shapes, sh = _get_callable(nc)
        concat_in = [
            np.concatenate([np.asarray(in_maps[c][n]) for c in range(N_CORES)],
                           axis=0) for n in in_names]
        concat_zeros = [np.zeros((N_CORES * s[0], *s[1:]), d)
                        for s, d in zero_shapes]
        dev_in = [jax.device_put(a, sh) for a in concat_in]
        dev_zero = [jax.device_put(a, sh) for a in concat_zeros]
        outs = fn(*dev_in, *dev_zero)
        arr = np.asarray(outs[out_names.index("out")])
        acc = arr.reshape(N_CORES, b_dim * t_dim, c_dim).astype(
            np.float32).sum(axis=0)
    except Exception:
        from concourse.bass_utils import run_bass_kernel_spmd
        res = run_bass_kernel_spmd(nc, in_maps, list(range(N_CORES)),
                                   trace=TRACE)
        global LAST_RESULT
        LAST_RESULT = res
        acc = np.zeros((b_dim * t_dim, c_dim), dtype=np.float32)
        for h in range(N_CORES):
            acc += res.results[h]["out"].astype(np.float32)
    acc *= OUT_DESCALE    # device partials stay scaled by SY*SWO
    return acc.reshape(b_dim, t_dim, c_dim)

